# revision 7
# baseline (speedup 1.0000x reference)
"""Trainium2 Bass kernel for causal average pooling (downsampling).

Reference op: out[b, i, d] = mean(x[b, :(i+1)*4, d]) over the time axis,
for x of shape (8, 8192, 512) f32 -> out (8, 2048, 512) f32.

Strategy
--------
Data-parallel over batch: one batch per NeuronCore (8 cores), no
cross-core communication.

Per core the math is, for each channel d independently, a prefix sum
over time sampled every SF=4 steps, scaled by 1/(4(i+1)).  We lay the
data out as [channel partitions, time free-dim] (the host pre-transposes
each batch, which is pure data movement) and use the hardware prefix
scan `tensor_tensor_scan` on the vector engine:

    state = (data0[t] + state) + data1[t]

Feeding data0 = x[:, 0::2] and data1 = x[:, 1::2] gives the cumulative
sum over PAIRS in 4096 steps: cs2[:, j] = sum(x[:, :2j+2]).  Output i of
the reference needs sum(x[:, :4i+4]) = cs2[:, 2i+1], so a strided gather
of the odd columns times a precomputed 1/(4(i+1)) table finishes the job
in one tensor_tensor multiply per tile.  DMA (16 MiB in + 4 MiB out per
core) is the roofline; the DVE work (~27 us) hides under it.

Written in raw Bass (not Tile): the walrus build in this container
enforces at most ONE semaphore wait per hardware instruction, so all
cross-engine waits are emitted as standalone wait_ge ops.

Engine plan per core (x viewed as xT [512 chan, 8192 time], 4 chan tiles):
  SP (sync) ring:   recip load + 4 x-tile loads (double-buffered)
  ACT ring:         4 out-tile stores
  DVE:              per tile: pair-scan (4096 steps) then gather*recip
"""

import sys

if "/opt/trn_rl_repo" not in sys.path:
    sys.path.insert(0, "/opt/trn_rl_repo")

import numpy as np

import concourse.bass as bass
import concourse.mybir as mybir
from concourse.bass_utils import run_bass_kernel_spmd

P = 128           # SBUF partitions
SF = 4            # pooling factor
B, L, D = 8, 8192, 512
N_CORES = 8


def build_bass(d=D, length=L):
    half = length // 2
    out_len = length // SF
    n_ct = d // P
    assert d % P == 0 and length % (2 * SF) == 0

    nc = bass.Bass()
    xT = nc.dram_tensor("xT", [d, length], mybir.dt.float32, kind="ExternalInput")
    recip = nc.dram_tensor(
        "recip", [P, out_len], mybir.dt.float32, kind="ExternalInput"
    )
    outT = nc.dram_tensor(
        "outT", [d, out_len], mybir.dt.float32, kind="ExternalOutput"
    )

    with (
        nc.sbuf_tensor([P, length], mybir.dt.float32) as xt0,
        nc.sbuf_tensor([P, length], mybir.dt.float32) as xt1,
        nc.sbuf_tensor([P, half], mybir.dt.float32) as cs0,
        nc.sbuf_tensor([P, half], mybir.dt.float32) as cs1,
        nc.sbuf_tensor([P, out_len], mybir.dt.float32) as rt,
        nc.sbuf_tensor([P, n_ct, out_len], mybir.dt.float32) as ot,
        nc.semaphore("s_rt") as s_rt,
        nc.semaphore("s_x0") as s_x0,
        nc.semaphore("s_x1") as s_x1,
        nc.semaphore("s_x2") as s_x2,
        nc.semaphore("s_x3") as s_x3,
        nc.semaphore("s_cmp") as s_cmp,
        nc.semaphore("s_out") as s_out,
        nc.Block() as block,
    ):
        xts = [xt0, xt1]
        css = [cs0, cs1]
        # One semaphore per load: completions of back-to-back DMAs on one
        # HWDGE ring are unordered, so a shared counting sem can't tell
        # which transfer actually landed.
        s_xs = [s_x0, s_x1, s_x2, s_x3][:n_ct]

        @block.sync
        def _(sync):
            # Loads on the SP HWDGE ring.
            sync.dma_start(out=rt[:, :], in_=recip[:, :]).then_inc(s_rt, 16)
            for ct in range(n_ct):
                if ct >= 2:
                    # xt slot WAR: scan ct-2 must be done with this buffer.
                    sync.wait_ge(s_cmp, 2 * (ct - 2) + 1)
                sync.dma_start(
                    out=xts[ct % 2][:, :], in_=xT[ct * P:(ct + 1) * P, :]
                ).then_inc(s_xs[ct], 16)

        @block.vector
        def _(vector):
            vector.wait_ge(s_rt, 16)
            for ct in range(n_ct):
                vector.wait_ge(s_xs[ct], 16)
                if ct >= 2:
                    # cs slot WAR vs mul ct-2; trivially satisfied by DVE
                    # program order, stated for the race checker.
                    vector.wait_ge(s_cmp, 2 * (ct - 2) + 2)
                xv = xts[ct % 2][:, :].rearrange("p (t two) -> p t two", two=2)
                nc.vector.tensor_tensor_scan(
                    css[ct % 2][:, :],
                    xv[:, :, 0],
                    xv[:, :, 1],
                    0.0,
                    mybir.AluOpType.add,
                    mybir.AluOpType.add,
                ).then_inc(s_cmp, 1)
                # scan -> mul RAW on the same engine; explicit for the checker.
                vector.wait_ge(s_cmp, 2 * ct + 1)
                csv = css[ct % 2][:, :].rearrange("p (t two) -> p t two", two=2)
                nc.vector.tensor_mul(
                    ot[:, ct, :], csv[:, :, 1], rt[:, :]
                ).then_inc(s_cmp, 1)

        @block.scalar
        def _(scalar):
            # Stores on the ACT HWDGE ring (independent of the load ring).
            for ct in range(n_ct):
                scalar.wait_ge(s_cmp, 2 * ct + 2)  # mul ct done
                scalar.dma_start(
                    out=outT[ct * P:(ct + 1) * P, :], in_=ot[:, ct, :]
                ).then_inc(s_out, 16)
            # Outputs must be in HBM before the kernel exits.
            scalar.wait_ge(s_out, 16 * n_ct)

    return nc


def _recip_table(out_len):
    r = 1.0 / (SF * np.arange(1, out_len + 1, dtype=np.float64))
    return np.broadcast_to(r.astype(np.float32), (P, out_len)).copy()


def kernel(x: np.ndarray) -> np.ndarray:
    b, length, d = x.shape
    out_len = length // SF
    # One batch per core, channels on partitions: host-side transpose is
    # pure layout so every DMA in the kernel is contiguous.
    xT = np.ascontiguousarray(np.swapaxes(np.asarray(x, dtype=np.float32), 1, 2))
    recip = _recip_table(out_len)
    in_maps = [{"xT": xT[i], "recip": recip} for i in range(b)]
    nc = build_bass(d=d, length=length)
    res = run_bass_kernel_spmd(nc, in_maps, core_ids=list(range(b)))
    outT = np.stack([res.results[i]["outT"] for i in range(b)])
    return np.ascontiguousarray(np.swapaxes(outT, 1, 2))
